# revision 9
# baseline (speedup 1.0000x reference)
"""Trainium2 Bass kernel for nn_BankedMergeHeads.

Math (per token t, slot k):
    out[t] = sum_k p[t,k] * (x[t,k] @ W[sel[t,k]] + b[sel[t,k]])

Strategy (8 NeuronCores = 4-way d_model x 2-way tokens):
  - Each core owns a (token-half, d_model-quarter): 1024 tokens x 512 cols.
  - Host-side routing ("dispatch"): sort the core's 4096 (token,slot) pairs
    by selected bank, pad each bank segment to a uniform per-bank capacity C
    (same C on all cores -> identical SPMD kernel IR; data-dependence lives
    only in input arrays).
  - Device grouped GEMM (orientation A): stationary = X^T piece (<=128
    pairs), moving = W[bank] column-slice (128x512), PSUM out = proj
    (pairs x 512) fp32. Per-bank input tiles so the GEMM pipelines with the
    input DMAs.
  - Evict PSUM -> SBUF fp16 (split across ScalarE/VectorE), DMA to a DRAM
    staging buffer (contiguous <=128-row blocks).
  - dma_gather permutes staging rows into token-major order (the MoE
    "combine" all-to-all, on-device). Descriptor generation is issued
    prepare_only during the GEMM phase and triggered once staging lands;
    the gather is split per token-chunk so the merge overlaps the drain.
  - Merge = PE matmuls: per 128-token chunk, PSUM accumulates one bias
    matmul (routing matrix PB^T @ b-slice) + four block-diagonal
    probability matmuls (P4^T @ gathered rows) via tile_position col tiles.
    The probabilities p live in the host-built routing matrices P4/PB
    (routing metadata x gate values); all x/W/b arithmetic is on-device.
  - Evict fp32 out chunks, DMA out; host reassembles the full output.
"""

import sys

import numpy as np

sys.path.insert(0, "/opt/trn_rl_repo")

# Problem constants (hardcoded per task contract).
B, S, K = 2, 1024, 4
NUM_BANKS = 32
D_HEAD = 128
D_MODEL = 2048
N_CORES = 8
DM, TK = 4, 2                # d_model split x token split
NT_L = (B * S) // TK         # tokens per core (1024)
NP_L = NT_L * K              # pairs per core (4096)
DMC = D_MODEL // DM          # cols per core (512)
NG = NT_L // 32              # 32-token merge groups per core
TCH = NT_L // 128            # 128-token chunks per core

_CACHE = {}

DEFAULT_GEMM_DTYPE = "f32r"


def _build_nc(C, gemm_dtype_name, split_gather=True, use_prep=True):
    """Build the SPMD Bass kernel. C = per-bank padded capacity (mult of 32)."""
    import concourse.bacc as bacc
    import concourse.mybir as mybir
    import concourse.tile as tile

    f32 = mybir.dt.float32
    f32r = mybir.dt.float32r
    fp16 = mybir.dt.float16
    i16 = mybir.dt.int16
    gdt = {"f32r": f32r, "fp16": fp16, "f32": f32}[gemm_dtype_name]

    NPAD = NUM_BANKS * C                    # padded pair rows
    GSPLIT = TCH if split_gather else 1     # gathers (per token chunk)
    IDX_PER = NP_L // GSPLIT                # idxs per gather

    nc = bacc.Bacc("TRN2", target_bir_lowering=False, debug=False,
                   num_devices=N_CORES)
    XT_d = nc.dram_tensor("XT", [D_HEAD, NPAD], gdt, kind="ExternalInput")
    W_d = nc.dram_tensor("Wq", [D_HEAD, NUM_BANKS * DMC], gdt,
                         kind="ExternalInput")
    gidx_d = nc.dram_tensor("gidx", [GSPLIT, 128, IDX_PER // 16], i16,
                            kind="ExternalInput")
    P4_d = nc.dram_tensor("P4", [128, NG * 32], fp16, kind="ExternalInput")
    PB_d = nc.dram_tensor("PB", [NUM_BANKS, NT_L], fp16, kind="ExternalInput")
    bT_d = nc.dram_tensor("bT", [NUM_BANKS, DMC], fp16, kind="ExternalInput")
    out_d = nc.dram_tensor("out", [NT_L, DMC], f32, kind="ExternalOutput")
    scratch_d = nc.dram_tensor("scratch", [NPAD, DMC], fp16)

    with tile.TileContext(nc) as tc:
        with tc.tile_pool(name="inp", bufs=1) as inp, \
             tc.tile_pool(name="ppg", bufs=5, space="PSUM") as ppg, \
             tc.tile_pool(name="ppm", bufs=3, space="PSUM") as ppm, \
             tc.tile_pool(name="ev", bufs=6) as ev, \
             tc.tile_pool(name="big", bufs=1) as big, \
             tc.tile_pool(name="ob", bufs=4) as ob:
            # small metadata first
            gidx_t = []
            for gsp in range(GSPLIT):
                gx = inp.tile([128, IDX_PER // 16], i16, tag=f"gx{gsp}")
                nc.sync.dma_start(gx[:], gidx_d.ap()[gsp])
                gidx_t.append(gx)
            P4 = inp.tile([128, NG * 32], fp16)
            nc.sync.dma_start(P4[:], P4_d.ap())
            PB = inp.tile([NUM_BANKS, NT_L], fp16)
            nc.sync.dma_start(PB[:], PB_d.ap())
            bT = inp.tile([NUM_BANKS, DMC], fp16)
            nc.sync.dma_start(bT[:], bT_d.ap())

            # prepare gather descriptors early (only needs gidx); the DMA
            # fires at trigger time after staging lands.
            merged_t = []
            for gsp in range(GSPLIT):
                mg = big.tile([128, IDX_PER // 128, DMC], fp16,
                              tag=f"mg{gsp}")
                merged_t.append(mg)
                if use_prep:
                    dma_sem = nc.alloc_semaphore(f"swdge_dma{gsp}")
                    nc.gpsimd.dma_gather(
                        out_ap=mg[:], in_ap=scratch_d.ap(),
                        idxs_ap=gidx_t[gsp][:],
                        num_idxs=IDX_PER, num_idxs_reg=IDX_PER, elem_size=DMC,
                        single_packet=False, prepare_only=True, sem=dma_sem)

            # per-bank input tiles -> precise deps, GEMM pipelines with DMA
            XT_t, W_t = [], []
            for n in range(NUM_BANKS):
                xt = inp.tile([D_HEAD, C], gdt, tag=f"xt{n}")
                nc.sync.dma_start(xt[:], XT_d.ap()[:, n * C:(n + 1) * C])
                w = inp.tile([D_HEAD, DMC], gdt, tag=f"w{n}")
                nc.sync.dma_start(w[:], W_d.ap()[:, n * DMC:(n + 1) * DMC])
                XT_t.append(xt)
                W_t.append(w)

            # ---- grouped GEMM + evict + stage-out ----
            evict_flip = 0
            for n in range(NUM_BANKS):
                off = 0
                while off < C:
                    m = min(128, C - off)
                    ps = ppg.tile([128, DMC], f32, tag="ps")
                    nc.tensor.matmul(
                        ps[:m, :], lhsT=XT_t[n][:, off:off + m],
                        rhs=W_t[n][:], start=True, stop=True)
                    st = ev.tile([128, DMC], fp16, tag="st")
                    if evict_flip == 0:
                        nc.scalar.copy(st[:m, :], ps[:m, :])
                    else:
                        nc.vector.tensor_copy(st[:m, :], ps[:m, :])
                    evict_flip ^= 1
                    nc.sync.dma_start(
                        scratch_d.ap()[n * C + off: n * C + off + m, :],
                        st[:m, :])
                    off += m

            # ---- fire the permutation DMAs ----
            if use_prep:
                nc.gpsimd.trigger_dma(count=None)
            else:
                for gsp in range(GSPLIT):
                    nc.gpsimd.dma_gather(
                        out_ap=merged_t[gsp][:], in_ap=scratch_d.ap(),
                        idxs_ap=gidx_t[gsp][:],
                        num_idxs=IDX_PER, num_idxs_reg=IDX_PER, elem_size=DMC,
                        single_packet=False)

            # ---- merge: bias matmul + 4 prob matmuls per 128-token chunk ----
            for t in range(TCH):
                po = ppm.tile([128, DMC], f32, tag="po")
                nc.tensor.matmul(
                    po[:], lhsT=PB[:, t * 128:(t + 1) * 128], rhs=bT[:],
                    start=True, stop=False)
                for j in range(4):
                    g = t * 4 + j
                    nc.tensor.matmul(
                        po[32 * j:32 * (j + 1), :],
                        lhsT=P4[:, g * 32:(g + 1) * 32],
                        rhs=(merged_t[t][:, j, :] if split_gather
                             else merged_t[0][:, g, :]),
                        start=False, stop=(j == 3),
                        tile_position=(0, 32 * j))
                osb = ob.tile([128, DMC], f32, tag="osb")
                if t % 2 == 0:
                    nc.scalar.copy(osb[:], po[:])
                else:
                    nc.vector.tensor_copy(osb[:], po[:])
                nc.sync.dma_start(out_d.ap()[t * 128:(t + 1) * 128, :], osb[:])

    nc.compile()
    return nc


def _prepare(tensor, head_selection, head_probabilities, W, b, C=None,
             gemm_dtype_name=DEFAULT_GEMM_DTYPE, split_gather=True):
    """Host-side sharding + routing metadata. Returns (in_maps, C)."""
    x = np.asarray(tensor, dtype=np.float32).reshape(B * S, K, D_HEAD)
    sel = np.asarray(head_selection).astype(np.int64).reshape(B * S, K)
    p = np.asarray(head_probabilities, dtype=np.float32).reshape(B * S, K)
    Wf = np.asarray(W, dtype=np.float32)
    bf = np.asarray(b, dtype=np.float32)

    halves = []
    maxcount = 0
    for tk in range(TK):
        t0 = tk * NT_L
        sel_h = sel[t0:t0 + NT_L].reshape(-1)          # (NP_L,)
        order = np.argsort(sel_h, kind="stable")        # sorted pair ids
        counts = np.bincount(sel_h, minlength=NUM_BANKS)
        maxcount = max(maxcount, int(counts.max()))
        halves.append((t0, sel_h, order, counts))
    if C is None:
        C = max(160, ((maxcount + 31) // 32) * 32)
    assert C >= maxcount
    NPAD = NUM_BANKS * C
    GSPLIT = TCH if split_gather else 1
    IDX_PER = NP_L // GSPLIT

    xdt = np.float16 if gemm_dtype_name == "fp16" else np.float32

    in_maps = [None] * N_CORES
    for tk in range(TK):
        t0, sel_h, order, counts = halves[tk]
        x_h = x[t0:t0 + NT_L].reshape(NP_L, D_HEAD)
        p_h = p[t0:t0 + NT_L]                          # (NT_L, K)

        # padded row position of each sorted pair
        seg_start = np.zeros(NUM_BANKS, dtype=np.int64)
        seg_start[1:] = np.cumsum(counts)[:-1]
        padpos = np.empty(NP_L, dtype=np.int64)
        for n in range(NUM_BANKS):
            s0 = seg_start[n]
            padpos[s0:s0 + counts[n]] = n * C + np.arange(counts[n])
        # pair j (token-major) -> padded row
        row_of_pair = np.empty(NP_L, dtype=np.int64)
        row_of_pair[order] = padpos

        Xpad = np.zeros((NPAD, D_HEAD), dtype=np.float32)
        Xpad[padpos] = x_h[order]
        XT_np = np.ascontiguousarray(Xpad.T).astype(xdt)   # (128, NPAD)

        gidx_np = np.zeros((GSPLIT, 128, IDX_PER // 16), dtype=np.int16)
        rof = row_of_pair.reshape(GSPLIT, IDX_PER)
        for gsp in range(GSPLIT):
            rows16 = rof[gsp].reshape(IDX_PER // 16, 16).T.astype(np.int16)
            for g in range(8):
                gidx_np[gsp, 16 * g:16 * (g + 1), :] = rows16

        P4_np = np.zeros((128, NG * 32), dtype=np.float16)
        pg = p_h.reshape(NG, 32, K)
        ii = np.arange(32)
        for gi in range(NG):
            for k in range(K):
                P4_np[4 * ii + k, gi * 32 + ii] = pg[gi, :, k]

        PB_np = np.zeros((NUM_BANKS, NT_L), dtype=np.float32)
        np.add.at(PB_np, (sel_h.reshape(NT_L, K).T,
                          np.broadcast_to(np.arange(NT_L), (K, NT_L))),
                  p_h.T)
        PB_np = PB_np.astype(np.float16)

        for dm in range(DM):
            c0 = dm * DMC
            W_np = np.ascontiguousarray(
                Wf[:, :, c0:c0 + DMC].transpose(1, 0, 2).reshape(
                    D_HEAD, NUM_BANKS * DMC)).astype(xdt)
            bT_np = bf[:, c0:c0 + DMC].astype(np.float16)
            core = tk * DM + dm
            in_maps[core] = {
                "XT": XT_np, "Wq": W_np, "gidx": gidx_np,
                "P4": P4_np, "PB": PB_np, "bT": bT_np,
            }
    return in_maps, C


def _run(tensor, head_selection, head_probabilities, W, b,
         gemm_dtype_name=DEFAULT_GEMM_DTYPE, split_gather=True, use_prep=True,
         trace=False):
    from concourse import bass_utils

    in_maps, C = _prepare(tensor, head_selection, head_probabilities, W, b,
                          gemm_dtype_name=gemm_dtype_name,
                          split_gather=split_gather)
    key = (C, gemm_dtype_name, split_gather, use_prep)
    if key not in _CACHE:
        _CACHE[key] = _build_nc(C, gemm_dtype_name, split_gather, use_prep)
    nc = _CACHE[key]
    res = bass_utils.run_bass_kernel_spmd(
        nc, in_maps, core_ids=list(range(N_CORES)), trace=trace)

    out = np.zeros((B * S, D_MODEL), dtype=np.float32)
    for core in range(N_CORES):
        tk, dm = core // DM, core % DM
        oc = res.results[core]["out"]
        out[tk * NT_L:(tk + 1) * NT_L, dm * DMC:(dm + 1) * DMC] = oc
    return out.reshape(B, S, D_MODEL), res


def kernel(tensor, head_selection, head_probabilities, W, b):
    out, _ = _run(tensor, head_selection, head_probabilities, W, b)
    return out


# revision 12
# speedup vs baseline: 1.4591x; 1.4591x over previous
"""Trainium2 Bass kernel for nn_BankedMergeHeads.

Math (per token t, slot k):
    out[t] = sum_k p[t,k] * (x[t,k] @ W[sel[t,k]] + b[sel[t,k]])

Strategy (8 NeuronCores = 4-way d_model x 2-way tokens):
  - Each core owns a (token-half, d_model-quarter): 1024 tokens x 512 cols.
  - Host-side routing ("dispatch"): sort the core's 4096 (token,slot) pairs
    by selected bank, pad each bank segment to a uniform per-bank capacity C
    (same C on all cores -> identical SPMD kernel IR; data-dependence lives
    only in input arrays).
  - Device grouped GEMM (orientation A): stationary = X^T piece (<=128
    pairs), moving = W[bank] column-slice (128x512), PSUM out = proj
    (pairs x 512) fp32. Per-bank input tiles so the GEMM pipelines with the
    input DMAs.
  - Evict PSUM -> SBUF fp16 (split across ScalarE/VectorE), DMA to a DRAM
    staging buffer (contiguous <=128-row blocks).
  - dma_gather permutes staging rows into token-major order (the MoE
    "combine" all-to-all, on-device). Descriptor generation is issued
    prepare_only during the GEMM phase and triggered once staging lands;
    the gather is split per token-chunk so the merge overlaps the drain.
  - Merge = PE matmuls: per 128-token chunk, PSUM accumulates one bias
    matmul (routing matrix PB^T @ b-slice) + four block-diagonal
    probability matmuls (P4^T @ gathered rows) via tile_position col tiles.
    The probabilities p live in the host-built routing matrices P4/PB
    (routing metadata x gate values); all x/W/b arithmetic is on-device.
  - Evict fp32 out chunks, DMA out; host reassembles the full output.
"""

import sys

import numpy as np

sys.path.insert(0, "/opt/trn_rl_repo")

# Problem constants (hardcoded per task contract).
B, S, K = 2, 1024, 4
NUM_BANKS = 32
D_HEAD = 128
D_MODEL = 2048
N_CORES = 8
DM, TK = 4, 2                # d_model split x token split
NT_L = (B * S) // TK         # tokens per core (1024)
NP_L = NT_L * K              # pairs per core (4096)
DMC = D_MODEL // DM          # cols per core (512)
NG = NT_L // 32              # 32-token merge groups per core
TCH = NT_L // 128            # 128-token chunks per core

_CACHE = {}

DEFAULT_GEMM_DTYPE = "f32r"


def _build_nc(C, gemm_dtype_name, split_gather=True, use_prep=True):
    """Build the SPMD Bass kernel. C = per-bank padded capacity (mult of 32)."""
    import concourse.bacc as bacc
    import concourse.mybir as mybir
    import concourse.tile as tile

    f32 = mybir.dt.float32
    f32r = mybir.dt.float32r
    fp16 = mybir.dt.float16
    i16 = mybir.dt.int16
    gdt = {"f32r": f32r, "fp16": fp16, "f32": f32}[gemm_dtype_name]

    NPAD = NUM_BANKS * C                    # padded pair rows
    GSPLIT = TCH if split_gather else 1     # gathers (per token chunk)
    IDX_PER = NP_L // GSPLIT                # idxs per gather

    nc = bacc.Bacc("TRN2", target_bir_lowering=False, debug=False,
                   num_devices=N_CORES)
    XT_d = nc.dram_tensor("XT", [D_HEAD, NPAD], gdt, kind="ExternalInput")
    W_d = nc.dram_tensor("Wq", [D_HEAD, NUM_BANKS * DMC], gdt,
                         kind="ExternalInput")
    gidx_d = nc.dram_tensor("gidx", [GSPLIT, 128, IDX_PER // 16], i16,
                            kind="ExternalInput")
    P4_d = nc.dram_tensor("P4", [128, NG * 32], fp16, kind="ExternalInput")
    PB_d = nc.dram_tensor("PB", [NUM_BANKS, NT_L], fp16, kind="ExternalInput")
    bT_d = nc.dram_tensor("bT", [NUM_BANKS, DMC], fp16, kind="ExternalInput")
    out_d = nc.dram_tensor("out", [NT_L, DMC], f32, kind="ExternalOutput")
    PPB_ = (C + 127) // 128
    scratch_d = nc.dram_tensor("scratch", [NUM_BANKS * PPB_ * 128, DMC], fp16)

    with tile.TileContext(nc) as tc:
        with tc.tile_pool(name="inp", bufs=1) as inp, \
             tc.tile_pool(name="ppg", bufs=5, space="PSUM") as ppg, \
             tc.tile_pool(name="ppm", bufs=3, space="PSUM") as ppm, \
             tc.tile_pool(name="ev", bufs=6) as ev, \
             tc.tile_pool(name="big", bufs=1) as big, \
             tc.tile_pool(name="ob", bufs=4) as ob:
            # small metadata first
            gidx_t = []
            for gsp in range(GSPLIT):
                gx = inp.tile([128, IDX_PER // 16], i16, tag=f"gx{gsp}")
                nc.sync.dma_start(gx[:], gidx_d.ap()[gsp])
                gidx_t.append(gx)
            P4 = inp.tile([128, NG * 32], fp16)
            nc.sync.dma_start(P4[:], P4_d.ap())
            PB = inp.tile([NUM_BANKS, NT_L], fp16)
            nc.sync.dma_start(PB[:], PB_d.ap())
            bT = inp.tile([NUM_BANKS, DMC], fp16)
            nc.sync.dma_start(bT[:], bT_d.ap())

            # prepare gather descriptors early (only needs gidx); the DMA
            # fires at trigger time after staging lands.
            merged_t = []
            for gsp in range(GSPLIT):
                mg = big.tile([128, IDX_PER // 128, DMC], fp16,
                              tag=f"mg{gsp}")
                merged_t.append(mg)
                if use_prep:
                    dma_sem = nc.alloc_semaphore(f"swdge_dma{gsp}")
                    nc.gpsimd.dma_gather(
                        out_ap=mg[:], in_ap=scratch_d.ap(),
                        idxs_ap=gidx_t[gsp][:],
                        num_idxs=IDX_PER, num_idxs_reg=IDX_PER, elem_size=DMC,
                        single_packet=False, prepare_only=True, sem=dma_sem)

            # bank-group input tiles -> GEMM pipelines with input DMA while
            # keeping the DMA instruction count low (SP issue ~0.6us each)
            GB = 4                      # banks per input DMA
            XT_t, W_t = [], []
            for n0 in range(0, NUM_BANKS, GB):
                xt = inp.tile([D_HEAD, GB * C], gdt, tag=f"xt{n0}")
                nc.sync.dma_start(xt[:], XT_d.ap()[:, n0 * C:(n0 + GB) * C])
                w = inp.tile([D_HEAD, GB * DMC], gdt, tag=f"w{n0}")
                nc.sync.dma_start(w[:], W_d.ap()[:, n0 * DMC:(n0 + GB) * DMC])
                for i in range(GB):
                    XT_t.append(xt[:, i * C:(i + 1) * C])
                    W_t.append(w[:, i * DMC:(i + 1) * DMC])

            # ---- grouped GEMM + evict + stage-out ----
            # each <=128-row piece gets its own 128-row slot in scratch so
            # evictions stay partition-0 aligned; stage-out is batched
            PPB = (C + 127) // 128           # pieces (slots) per bank
            NSLOT = NUM_BANKS * PPB
            GS = 8                           # slots per stage-out DMA
            assert NSLOT % GS == 0
            evict_flip = 0
            st = None
            for n in range(NUM_BANKS):
                for k in range(PPB):
                    slot = n * PPB + k
                    off = k * 128
                    m = min(128, C - off)
                    if slot % GS == 0:
                        st = ev.tile([128, GS, DMC], fp16, tag="st")
                    ps = ppg.tile([128, DMC], f32, tag="ps")
                    nc.tensor.matmul(
                        ps[:m, :], lhsT=XT_t[n][:, off:off + m],
                        rhs=W_t[n], start=True, stop=True)
                    g = slot % GS
                    if evict_flip == 0:
                        nc.scalar.copy(st[:m, g, :], ps[:m, :])
                    else:
                        nc.vector.tensor_copy(st[:m, g, :], ps[:m, :])
                    evict_flip ^= 1
                    if g == GS - 1:
                        s0 = slot - GS + 1
                        nc.sync.dma_start(
                            scratch_d.ap().rearrange(
                                "(b p) m -> p b m", p=128)[:, s0:s0 + GS, :],
                            st[:])

            # ---- fire the permutation DMAs ----
            if use_prep:
                nc.gpsimd.trigger_dma(count=None)
            else:
                for gsp in range(GSPLIT):
                    nc.gpsimd.dma_gather(
                        out_ap=merged_t[gsp][:], in_ap=scratch_d.ap(),
                        idxs_ap=gidx_t[gsp][:],
                        num_idxs=IDX_PER, num_idxs_reg=IDX_PER, elem_size=DMC,
                        single_packet=False)

            # ---- merge: bias matmul + 4 prob matmuls per 128-token chunk ----
            for t in range(TCH):
                po = ppm.tile([128, DMC], f32, tag="po")
                nc.tensor.matmul(
                    po[:], lhsT=PB[:, t * 128:(t + 1) * 128], rhs=bT[:],
                    start=True, stop=False)
                for j in range(4):
                    g = t * 4 + j
                    nc.tensor.matmul(
                        po[32 * j:32 * (j + 1), :],
                        lhsT=P4[:, g * 32:(g + 1) * 32],
                        rhs=(merged_t[t][:, j, :] if split_gather
                             else merged_t[0][:, g, :]),
                        start=False, stop=(j == 3),
                        tile_position=(0, 32 * j))
                osb = ob.tile([128, DMC], f32, tag="osb")
                if t % 2 == 0:
                    nc.scalar.copy(osb[:], po[:])
                else:
                    nc.vector.tensor_copy(osb[:], po[:])
                nc.sync.dma_start(out_d.ap()[t * 128:(t + 1) * 128, :], osb[:])

    nc.compile()
    return nc


def _prepare(tensor, head_selection, head_probabilities, W, b, C=None,
             gemm_dtype_name=DEFAULT_GEMM_DTYPE, split_gather=True):
    """Host-side sharding + routing metadata. Returns (in_maps, C)."""
    x = np.asarray(tensor, dtype=np.float32).reshape(B * S, K, D_HEAD)
    sel = np.asarray(head_selection).astype(np.int64).reshape(B * S, K)
    p = np.asarray(head_probabilities, dtype=np.float32).reshape(B * S, K)
    Wf = np.asarray(W, dtype=np.float32)
    bf = np.asarray(b, dtype=np.float32)

    halves = []
    maxcount = 0
    for tk in range(TK):
        t0 = tk * NT_L
        sel_h = sel[t0:t0 + NT_L].reshape(-1)          # (NP_L,)
        order = np.argsort(sel_h, kind="stable")        # sorted pair ids
        counts = np.bincount(sel_h, minlength=NUM_BANKS)
        maxcount = max(maxcount, int(counts.max()))
        halves.append((t0, sel_h, order, counts))
    if C is None:
        C = max(160, ((maxcount + 31) // 32) * 32)
    assert C >= maxcount
    NPAD = NUM_BANKS * C
    GSPLIT = TCH if split_gather else 1
    IDX_PER = NP_L // GSPLIT

    xdt = np.float16 if gemm_dtype_name == "fp16" else np.float32

    in_maps = [None] * N_CORES
    for tk in range(TK):
        t0, sel_h, order, counts = halves[tk]
        x_h = x[t0:t0 + NT_L].reshape(NP_L, D_HEAD)
        p_h = p[t0:t0 + NT_L]                          # (NT_L, K)

        # padded row position of each sorted pair
        seg_start = np.zeros(NUM_BANKS, dtype=np.int64)
        seg_start[1:] = np.cumsum(counts)[:-1]
        padpos = np.empty(NP_L, dtype=np.int64)
        PPB = (C + 127) // 128
        scrow = np.empty(NP_L, dtype=np.int64)    # scratch slot-row
        for n in range(NUM_BANKS):
            s0 = seg_start[n]
            i = np.arange(counts[n])
            padpos[s0:s0 + counts[n]] = n * C + i
            scrow[s0:s0 + counts[n]] = (n * PPB + i // 128) * 128 + i % 128
        # pair j (token-major) -> scratch row
        row_of_pair = np.empty(NP_L, dtype=np.int64)
        row_of_pair[order] = scrow

        Xpad = np.zeros((NPAD, D_HEAD), dtype=np.float32)
        Xpad[padpos] = x_h[order]
        XT_np = np.ascontiguousarray(Xpad.T).astype(xdt)   # (128, NPAD)

        gidx_np = np.zeros((GSPLIT, 128, IDX_PER // 16), dtype=np.int16)
        rof = row_of_pair.reshape(GSPLIT, IDX_PER)
        for gsp in range(GSPLIT):
            rows16 = rof[gsp].reshape(IDX_PER // 16, 16).T.astype(np.int16)
            for g in range(8):
                gidx_np[gsp, 16 * g:16 * (g + 1), :] = rows16

        P4_np = np.zeros((128, NG * 32), dtype=np.float16)
        pg = p_h.reshape(NG, 32, K)
        ii = np.arange(32)
        for gi in range(NG):
            for k in range(K):
                P4_np[4 * ii + k, gi * 32 + ii] = pg[gi, :, k]

        PB_np = np.zeros((NUM_BANKS, NT_L), dtype=np.float32)
        np.add.at(PB_np, (sel_h.reshape(NT_L, K).T,
                          np.broadcast_to(np.arange(NT_L), (K, NT_L))),
                  p_h.T)
        PB_np = PB_np.astype(np.float16)

        for dm in range(DM):
            c0 = dm * DMC
            W_np = np.ascontiguousarray(
                Wf[:, :, c0:c0 + DMC].transpose(1, 0, 2).reshape(
                    D_HEAD, NUM_BANKS * DMC)).astype(xdt)
            bT_np = bf[:, c0:c0 + DMC].astype(np.float16)
            core = tk * DM + dm
            in_maps[core] = {
                "XT": XT_np, "Wq": W_np, "gidx": gidx_np,
                "P4": P4_np, "PB": PB_np, "bT": bT_np,
            }
    return in_maps, C


def _run(tensor, head_selection, head_probabilities, W, b,
         gemm_dtype_name=DEFAULT_GEMM_DTYPE, split_gather=True, use_prep=True,
         trace=False):
    from concourse import bass_utils

    in_maps, C = _prepare(tensor, head_selection, head_probabilities, W, b,
                          gemm_dtype_name=gemm_dtype_name,
                          split_gather=split_gather)
    key = (C, gemm_dtype_name, split_gather, use_prep)
    if key not in _CACHE:
        _CACHE[key] = _build_nc(C, gemm_dtype_name, split_gather, use_prep)
    nc = _CACHE[key]
    res = bass_utils.run_bass_kernel_spmd(
        nc, in_maps, core_ids=list(range(N_CORES)), trace=trace)

    out = np.zeros((B * S, D_MODEL), dtype=np.float32)
    for core in range(N_CORES):
        tk, dm = core // DM, core % DM
        oc = res.results[core]["out"]
        out[tk * NT_L:(tk + 1) * NT_L, dm * DMC:(dm + 1) * DMC] = oc
    return out.reshape(B, S, D_MODEL), res


def kernel(tensor, head_selection, head_probabilities, W, b):
    out, _ = _run(tensor, head_selection, head_probabilities, W, b)
    return out


# revision 14
# speedup vs baseline: 1.6118x; 1.1047x over previous
"""Trainium2 Bass kernel for nn_BankedMergeHeads.

Math (per token t, slot k):
    out[t] = sum_k p[t,k] * (x[t,k] @ W[sel[t,k]] + b[sel[t,k]])

Strategy (8 NeuronCores = 4-way d_model x 2-way tokens):
  - Each core owns a (token-half, d_model-quarter): 1024 tokens x 512 cols.
  - Host-side routing ("dispatch"): sort the core's 4096 (token,slot) pairs
    by selected bank, pad each bank segment to a uniform per-bank capacity C
    (same C on all cores -> identical SPMD kernel IR; data-dependence lives
    only in input arrays).
  - Device grouped GEMM (orientation A): stationary = X^T piece (<=128
    pairs), moving = W[bank] column-slice (128x512), PSUM out = proj
    (pairs x 512) fp32. Per-bank input tiles so the GEMM pipelines with the
    input DMAs.
  - Evict PSUM -> SBUF fp16 (split across ScalarE/VectorE), DMA to a DRAM
    staging buffer (contiguous <=128-row blocks).
  - dma_gather permutes staging rows into token-major order (the MoE
    "combine" all-to-all, on-device). Descriptor generation is issued
    prepare_only during the GEMM phase and triggered once staging lands;
    the gather is split per token-chunk so the merge overlaps the drain.
  - Merge = PE matmuls: per 128-token chunk, PSUM accumulates one bias
    matmul (routing matrix PB^T @ b-slice) + four block-diagonal
    probability matmuls (P4^T @ gathered rows) via tile_position col tiles.
    The probabilities p live in the host-built routing matrices P4/PB
    (routing metadata x gate values); all x/W/b arithmetic is on-device.
  - Evict fp32 out chunks, DMA out; host reassembles the full output.
"""

import sys

import numpy as np

sys.path.insert(0, "/opt/trn_rl_repo")

# Problem constants (hardcoded per task contract).
B, S, K = 2, 1024, 4
NUM_BANKS = 32
D_HEAD = 128
D_MODEL = 2048
N_CORES = 8
DM, TK = 4, 2                # d_model split x token split
NT_L = (B * S) // TK         # tokens per core (1024)
NP_L = NT_L * K              # pairs per core (4096)
DMC = D_MODEL // DM          # cols per core (512)
NG = NT_L // 32              # 32-token merge groups per core
TCH = NT_L // 128            # 128-token chunks per core

_CACHE = {}

DEFAULT_GEMM_DTYPE = "f32r"


def _build_nc(C, gemm_dtype_name, split_gather=True, use_prep=True):
    """Build the SPMD Bass kernel. C = per-bank padded capacity (mult of 32)."""
    import concourse.bacc as bacc
    import concourse.mybir as mybir
    import concourse.tile as tile

    f32 = mybir.dt.float32
    f32r = mybir.dt.float32r
    fp16 = mybir.dt.float16
    i16 = mybir.dt.int16
    gdt = {"f32r": f32r, "fp16": fp16, "f32": f32}[gemm_dtype_name]

    NPAD = NUM_BANKS * C                    # padded pair rows
    GSPLIT = TCH if split_gather else 1     # gathers (per token chunk)
    IDX_PER = NP_L // GSPLIT                # idxs per gather

    nc = bacc.Bacc("TRN2", target_bir_lowering=False, debug=False,
                   num_devices=N_CORES, num_swdge_queues=4)
    XT_d = nc.dram_tensor("XT", [D_HEAD, NPAD], gdt, kind="ExternalInput")
    W_d = nc.dram_tensor("Wq", [D_HEAD, NUM_BANKS * DMC], gdt,
                         kind="ExternalInput")
    gidx_d = nc.dram_tensor("gidx", [GSPLIT, 128, IDX_PER // 16], i16,
                            kind="ExternalInput")
    P4_d = nc.dram_tensor("P4", [128, NG * 32], fp16, kind="ExternalInput")
    PB_d = nc.dram_tensor("PB", [NUM_BANKS, NT_L], fp16, kind="ExternalInput")
    bT_d = nc.dram_tensor("bT", [NUM_BANKS, DMC], fp16, kind="ExternalInput")
    out_d = nc.dram_tensor("out", [NT_L, DMC], f32, kind="ExternalOutput")
    PPB_ = (C + 127) // 128
    scratch_d = nc.dram_tensor("scratch", [NUM_BANKS * PPB_ * 128, DMC], fp16)

    with tile.TileContext(nc) as tc:
        with tc.tile_pool(name="inp", bufs=1) as inp, \
             tc.tile_pool(name="ppg", bufs=5, space="PSUM") as ppg, \
             tc.tile_pool(name="ppm", bufs=3, space="PSUM") as ppm, \
             tc.tile_pool(name="ev", bufs=6) as ev, \
             tc.tile_pool(name="big", bufs=1) as big, \
             tc.tile_pool(name="ob", bufs=4) as ob:
            # small metadata first
            gidx_t = []
            for gsp in range(GSPLIT):
                gx = inp.tile([128, IDX_PER // 16], i16, tag=f"gx{gsp}")
                nc.sync.dma_start(gx[:], gidx_d.ap()[gsp])
                gidx_t.append(gx)
            P4 = inp.tile([128, NG * 32], fp16)
            nc.sync.dma_start(P4[:], P4_d.ap())
            PB = inp.tile([NUM_BANKS, NT_L], fp16)
            nc.sync.dma_start(PB[:], PB_d.ap())
            bT = inp.tile([NUM_BANKS, DMC], fp16)
            nc.sync.dma_start(bT[:], bT_d.ap())

            # prepare gather descriptors early (only needs gidx); the DMA
            # fires at trigger time after staging lands.
            merged_t = []
            for gsp in range(GSPLIT):
                mg = big.tile([128, IDX_PER // 128, DMC], fp16,
                              tag=f"mg{gsp}")
                merged_t.append(mg)

            # bank-group input tiles -> GEMM pipelines with input DMA while
            # keeping the DMA instruction count low (SP issue ~0.6us each)
            GB = 8                      # banks per input DMA
            XT_t, W_t = [], []
            for n0 in range(0, NUM_BANKS, GB):
                xt = inp.tile([D_HEAD, GB * C], gdt, tag=f"xt{n0}")
                nc.sync.dma_start(xt[:], XT_d.ap()[:, n0 * C:(n0 + GB) * C])
                w = inp.tile([D_HEAD, GB * DMC], gdt, tag=f"w{n0}")
                nc.sync.dma_start(w[:], W_d.ap()[:, n0 * DMC:(n0 + GB) * DMC])
                for i in range(GB):
                    XT_t.append(xt[:, i * C:(i + 1) * C])
                    W_t.append(w[:, i * DMC:(i + 1) * DMC])

            # ---- grouped GEMM + evict + stage-out ----
            # each <=128-row piece gets its own 128-row slot in scratch so
            # evictions stay partition-0 aligned; stage-out is batched
            PPB = (C + 127) // 128           # pieces (slots) per bank
            NSLOT = NUM_BANKS * PPB
            GS = 8                           # slots per stage-out DMA
            assert NSLOT % GS == 0
            evict_flip = 0
            st = None
            for n in range(NUM_BANKS):
                for k in range(PPB):
                    slot = n * PPB + k
                    off = k * 128
                    m = min(128, C - off)
                    if slot % GS == 0:
                        st = ev.tile([128, GS, DMC], fp16, tag="st")
                    ps = ppg.tile([128, DMC], f32, tag="ps")
                    nc.tensor.matmul(
                        ps[:m, :], lhsT=XT_t[n][:, off:off + m],
                        rhs=W_t[n], start=True, stop=True)
                    g = slot % GS
                    if evict_flip == 0:
                        nc.scalar.copy(st[:m, g, :], ps[:m, :])
                    else:
                        nc.vector.tensor_copy(st[:m, g, :], ps[:m, :])
                    evict_flip ^= 1
                    if g == GS - 1:
                        s0 = slot - GS + 1
                        nc.sync.dma_start(
                            scratch_d.ap().rearrange(
                                "(b p) m -> p b m", p=128)[:, s0:s0 + GS, :],
                            st[:])

            # ---- fire the permutation DMAs ----
            if use_prep:
                for gsp in range(GSPLIT):
                    dma_sem = nc.alloc_semaphore(f"swdge_dma{gsp}")
                    nc.gpsimd.dma_gather(
                        out_ap=merged_t[gsp][:], in_ap=scratch_d.ap(),
                        idxs_ap=gidx_t[gsp][:],
                        num_idxs=IDX_PER, num_idxs_reg=IDX_PER, elem_size=DMC,
                        single_packet=False, prepare_only=True, sem=dma_sem)
                    nc.gpsimd.trigger_dma(count=None)
            else:
                for gsp in range(GSPLIT):
                    nc.gpsimd.dma_gather(
                        out_ap=merged_t[gsp][:], in_ap=scratch_d.ap(),
                        idxs_ap=gidx_t[gsp][:],
                        num_idxs=IDX_PER, num_idxs_reg=IDX_PER, elem_size=DMC,
                        single_packet=False, queue_num=gsp % 4)

            # ---- merge: bias matmul + 4 prob matmuls per 128-token chunk ----
            for t in range(TCH):
                po = ppm.tile([128, DMC], f32, tag="po")
                nc.tensor.matmul(
                    po[:], lhsT=PB[:, t * 128:(t + 1) * 128], rhs=bT[:],
                    start=True, stop=False)
                for j in range(4):
                    g = t * 4 + j
                    nc.tensor.matmul(
                        po[32 * j:32 * (j + 1), :],
                        lhsT=P4[:, g * 32:(g + 1) * 32],
                        rhs=(merged_t[t][:, j, :] if split_gather
                             else merged_t[0][:, g, :]),
                        start=False, stop=(j == 3),
                        tile_position=(0, 32 * j))
                if t % 4 == 0:
                    osb = ob.tile([128, 4, DMC], f32, tag="osb")
                if t % 2 == 0:
                    nc.scalar.copy(osb[:, t % 4, :], po[:])
                else:
                    nc.vector.tensor_copy(osb[:, t % 4, :], po[:])
                if t % 4 == 3:
                    t0_ = t - 3
                    nc.sync.dma_start(
                        out_d.ap().rearrange("(c p) m -> p c m", p=128)[
                            :, t0_:t0_ + 4, :],
                        osb[:])

    nc.compile()
    return nc


def _prepare(tensor, head_selection, head_probabilities, W, b, C=None,
             gemm_dtype_name=DEFAULT_GEMM_DTYPE, split_gather=True):
    """Host-side sharding + routing metadata. Returns (in_maps, C)."""
    x = np.asarray(tensor, dtype=np.float32).reshape(B * S, K, D_HEAD)
    sel = np.asarray(head_selection).astype(np.int64).reshape(B * S, K)
    p = np.asarray(head_probabilities, dtype=np.float32).reshape(B * S, K)
    Wf = np.asarray(W, dtype=np.float32)
    bf = np.asarray(b, dtype=np.float32)

    halves = []
    maxcount = 0
    for tk in range(TK):
        t0 = tk * NT_L
        sel_h = sel[t0:t0 + NT_L].reshape(-1)          # (NP_L,)
        order = np.argsort(sel_h, kind="stable")        # sorted pair ids
        counts = np.bincount(sel_h, minlength=NUM_BANKS)
        maxcount = max(maxcount, int(counts.max()))
        halves.append((t0, sel_h, order, counts))
    if C is None:
        C = max(160, ((maxcount + 31) // 32) * 32)
    assert C >= maxcount
    NPAD = NUM_BANKS * C
    GSPLIT = TCH if split_gather else 1
    IDX_PER = NP_L // GSPLIT

    xdt = np.float16 if gemm_dtype_name == "fp16" else np.float32

    in_maps = [None] * N_CORES
    for tk in range(TK):
        t0, sel_h, order, counts = halves[tk]
        x_h = x[t0:t0 + NT_L].reshape(NP_L, D_HEAD)
        p_h = p[t0:t0 + NT_L]                          # (NT_L, K)

        # padded row position of each sorted pair
        seg_start = np.zeros(NUM_BANKS, dtype=np.int64)
        seg_start[1:] = np.cumsum(counts)[:-1]
        padpos = np.empty(NP_L, dtype=np.int64)
        PPB = (C + 127) // 128
        scrow = np.empty(NP_L, dtype=np.int64)    # scratch slot-row
        for n in range(NUM_BANKS):
            s0 = seg_start[n]
            i = np.arange(counts[n])
            padpos[s0:s0 + counts[n]] = n * C + i
            scrow[s0:s0 + counts[n]] = (n * PPB + i // 128) * 128 + i % 128
        # pair j (token-major) -> scratch row
        row_of_pair = np.empty(NP_L, dtype=np.int64)
        row_of_pair[order] = scrow

        Xpad = np.zeros((NPAD, D_HEAD), dtype=np.float32)
        Xpad[padpos] = x_h[order]
        XT_np = np.ascontiguousarray(Xpad.T).astype(xdt)   # (128, NPAD)

        gidx_np = np.zeros((GSPLIT, 128, IDX_PER // 16), dtype=np.int16)
        rof = row_of_pair.reshape(GSPLIT, IDX_PER)
        for gsp in range(GSPLIT):
            rows16 = rof[gsp].reshape(IDX_PER // 16, 16).T.astype(np.int16)
            for g in range(8):
                gidx_np[gsp, 16 * g:16 * (g + 1), :] = rows16

        P4_np = np.zeros((128, NG * 32), dtype=np.float16)
        pg = p_h.reshape(NG, 32, K)
        ii = np.arange(32)
        for gi in range(NG):
            for k in range(K):
                P4_np[4 * ii + k, gi * 32 + ii] = pg[gi, :, k]

        PB_np = np.zeros((NUM_BANKS, NT_L), dtype=np.float32)
        np.add.at(PB_np, (sel_h.reshape(NT_L, K).T,
                          np.broadcast_to(np.arange(NT_L), (K, NT_L))),
                  p_h.T)
        PB_np = PB_np.astype(np.float16)

        for dm in range(DM):
            c0 = dm * DMC
            W_np = np.ascontiguousarray(
                Wf[:, :, c0:c0 + DMC].transpose(1, 0, 2).reshape(
                    D_HEAD, NUM_BANKS * DMC)).astype(xdt)
            bT_np = bf[:, c0:c0 + DMC].astype(np.float16)
            core = tk * DM + dm
            in_maps[core] = {
                "XT": XT_np, "Wq": W_np, "gidx": gidx_np,
                "P4": P4_np, "PB": PB_np, "bT": bT_np,
            }
    return in_maps, C


def _run(tensor, head_selection, head_probabilities, W, b,
         gemm_dtype_name=DEFAULT_GEMM_DTYPE, split_gather=True, use_prep=True,
         trace=False):
    from concourse import bass_utils

    in_maps, C = _prepare(tensor, head_selection, head_probabilities, W, b,
                          gemm_dtype_name=gemm_dtype_name,
                          split_gather=split_gather)
    key = (C, gemm_dtype_name, split_gather, use_prep)
    if key not in _CACHE:
        _CACHE[key] = _build_nc(C, gemm_dtype_name, split_gather, use_prep)
    nc = _CACHE[key]
    res = bass_utils.run_bass_kernel_spmd(
        nc, in_maps, core_ids=list(range(N_CORES)), trace=trace)

    out = np.zeros((B * S, D_MODEL), dtype=np.float32)
    for core in range(N_CORES):
        tk, dm = core // DM, core % DM
        oc = res.results[core]["out"]
        out[tk * NT_L:(tk + 1) * NT_L, dm * DMC:(dm + 1) * DMC] = oc
    return out.reshape(B, S, D_MODEL), res


def kernel(tensor, head_selection, head_probabilities, W, b):
    out, _ = _run(tensor, head_selection, head_probabilities, W, b)
    return out
